# revision 77
# baseline (speedup 1.0000x reference)
"""Local (sliding-window causal) attention on 8 Trainium2 NeuronCores.

Problem: B=1, S=4096, D=1024, H=16 heads (hd=64), WINDOW=256.
Sharding: tensor-parallel over heads -- 2 heads per core. Each core computes
q/k/v projections for its 2 heads, windowed softmax attention, and its
partial contribution o_c @ Wo_c. The host sums the 8 partials and adds the
bias terms.

Key design points (v2):
 - All matmuls in fp16 (1 cycle/row on the PE vs ~4.5 for fp32 HIGH mode);
   accumulation stays fp32 in PSUM. Host casts inputs to fp16.
 - Scores are computed TRANSPOSED: sT[key, q] = kT_block^T @ qT, so the
   exp'd probabilities feed the PV matmul (P as the stationary operand)
   directly -- no PE transposes of P.
 - Causal-window masking is multiplicative 0/1 AFTER exp (scores are small,
   |s| < 3, so exp never overflows). Per 128-query block the 384-key padded
   window splits into 3 key blocks of which only two need (triangular)
   masks; expm column order [kb1 | kb0 | kb2] makes them one contiguous
   multiply, split across the vector and gpsimd engines by head.
 - PV uses expm as the stationary operand so its output oa is [q, hd+1]
   with q on partitions; V blocks carry a 65th column of ones, so oa's last
   column is the softmax row sum r. Normalization is then a per-partition
   reciprocal + multiply, and one PE transpose restores [hd, q] for the
   output projection.
 - score scale 1/sqrt(hd) folded into Wq host-side; bq added on-device via
   per-partition tensor_scalar; bk dropped (softmax shift invariance);
   bv & bo contributions added host-side (softmax rows sum to 1).
 - 4-deep software pipeline over query blocks; DMA descriptor issue is
   spread across the sync/gpsimd/scalar queues (serial issue costs ~650ns
   each); y writeback is split per 512-col half, with a 4-way split on the
   last blocks to shorten the drain tail.
 - Keeping the per-block matmuls fine-grained is deliberate: a fused
   key-major variant with 3x fewer score matmuls measured SLOWER because
   the denser PE stream doubled power-throttle time (util limit 0.5).

Math notes:
 - score uses (q + bq) . (k + bk); the q.bk and bq.bk terms are constant per
   query row so they drop under softmax -> bk is dropped, bq folded into q.
 - v bias: o = p @ (v + bv) = p @ v + bv (softmax rows sum to 1), so the bv
   contribution to the output is the constant row bv @ Wo, added on host.
"""

import numpy as np

import concourse.bass as bass
import concourse.tile as tile
from concourse import bacc, mybir
from concourse.bass_utils import run_bass_kernel_spmd

# Problem constants (hardcoded per contract -- kernel.py must be self-contained)
S = 4096
D = 1024
H = 16
HD = 64
WINDOW = 256
N_CORES = 8
HPC = H // N_CORES          # heads per core = 2
DH = HPC * HD               # per-core head dims = 128

F16 = mybir.dt.float16
F32 = mybir.dt.float32
F32R = mybir.dt.float32r

N_QB = S // 128             # 32 query blocks (and key blocks)
N_T = S // 512              # 8 projection seq chunks
KC = D // 128               # 8 contraction chunks
VB = HD + 1                 # v block stride: 64 v columns + a ones column


def _make_mask():
    """Multiplicative masks [128, 256] fp16 in transposed [key, query] layout.

    Query block qb sees key blocks g = qb-2+kb (kb = 0,1,2): kb=0 allows
    local key jl > query qi, kb=1 allows all (no mask), kb=2 allows
    jl <= qi. The expm tiles use column order [kb1 | kb0 | kb2] so the two
    triangular masks land in one contiguous [128, 256] multiply.
    """
    jl = np.arange(128)[:, None]
    qi = np.arange(128)[None, :]
    m = np.ones((128, 256), dtype=np.float16)
    m[:, 0:128] = (jl > qi).astype(np.float16)
    m[:, 128:256] = (jl <= qi).astype(np.float16)
    return m


# expm/sc column offset per key block kb, in [kb1 | kb0 | kb2] order
COL = {0: 128, 1: 0, 2: 256}


def build_kernel():
    nc = bacc.Bacc()

    xT = nc.dram_tensor("xT", [D, S], F16, kind="ExternalInput")
    wq = nc.dram_tensor("wq", [D, DH], F16, kind="ExternalInput")
    wk = nc.dram_tensor("wk", [D, DH], F16, kind="ExternalInput")
    wv = nc.dram_tensor("wv", [D, DH], F16, kind="ExternalInput")
    bq = nc.dram_tensor("bq", [DH], F32, kind="ExternalInput")
    wo = nc.dram_tensor("wo", [DH, D], F16, kind="ExternalInput")
    y = nc.dram_tensor("y", [S, D], F16, kind="ExternalOutput")

    mask_d = nc.inline_tensor(_make_mask(), name="mask")
    ident_d = nc.inline_tensor(np.eye(128, dtype=np.float32), name="ident")

    with tile.TileContext(nc) as tc:
        with (
            nc.allow_low_precision(
                reason="fp16 activations by design; rel-err budget 2e-2"),
            tc.tile_pool(name="consts", bufs=1) as consts,
            tc.tile_pool(name="persist", bufs=1) as persist,
            tc.tile_pool(name="xstream", bufs=2) as xstream,
            tc.tile_pool(name="expp", bufs=4) as expp,
            tc.tile_pool(name="work", bufs=4) as work,
        ):
            # ---- constants to SBUF ----
            # First x chunk goes out on sync immediately -- its ~5.6us
            # transfer is the critical path to the first matmul. Constants
            # spread across the other queues meanwhile.
            xt0 = xstream.tile([128, KC, 512], F16, name="xt")
            for c in range(KC):
                # halves land on two queues: the first matmul's input arrives
                # in ~2.8us instead of ~5.6us
                nc.sync.dma_start(xt0[:, c, 0:256],
                                  xT.ap()[c * 128:(c + 1) * 128, 0:256])
                nc.gpsimd.dma_start(xt0[:, c, 256:512],
                                    xT.ap()[c * 128:(c + 1) * 128, 256:512])

            # Weight chunks must not queue behind the xt0 issues: wq goes on
            # the otherwise-empty scalar queue so the first projection matmul
            # is gated by the x transfer (~10us), not the weights.
            wq_t = consts.tile([128, KC * DH], F16, name="wq_t")
            wk_t = consts.tile([128, KC * DH], F16, name="wk_t")
            wv_t = consts.tile([128, KC * DH], F16, name="wv_t")
            for (t, d, eng) in ((wq_t, wq, nc.scalar), (wv_t, wv, nc.scalar),
                                (wk_t, wk, nc.gpsimd)):
                d3 = d.ap().rearrange("(c p) m -> p c m", p=128)
                for c in range(KC):
                    eng.dma_start(t[:, c * DH:(c + 1) * DH], d3[:, c])
            wo_t = consts.tile([DH, D], F16, name="wo_t")
            nc.scalar.dma_start(wo_t, wo.ap())

            bqs = consts.tile([DH, 1], F32, name="bqs")
            nc.scalar.dma_start(bqs, bq.ap().rearrange("(p o) -> p o", o=1))
            # needed only once attention starts (~+45us): last in line
            mask = consts.tile([128, 256], F16, name="mask")
            nc.scalar.dma_start(mask, mask_d.ap())
            ident = consts.tile([128, 128], F32R, name="ident")
            nc.scalar.dma_start(ident, ident_d.ap().bitcast(F32R))

            # ---- persistent activations ----
            qT = persist.tile([128, S], F16, name="qT")
            kT = persist.tile([128, S], F16, name="kT")
            vv = persist.tile([128, 2 * N_QB, VB], F16, name="vv")
            # ones columns for the rowsum rows of PV; v proj fills cols 0:64
            nc.vector.memset(vv[:, :, 64:65], 1.0)

            # ---- projections ----
            with (
                tc.tile_pool(name="proj_ps", bufs=2, space="PSUM") as proj_ps,
                tc.tile_pool(name="vt_ps", bufs=2, space="PSUM") as vt_ps,
            ):
                for t in range(N_T):
                    sl = slice(t * 512, (t + 1) * 512)
                    if t == 0:
                        xt = xt0
                    else:
                        xt = xstream.tile([128, KC, 512], F16, name="xt")
                        for c in range(KC):
                            (nc.sync if c % 2 == 0 else nc.gpsimd).dma_start(
                                xt[:, c], xT.ap()[c * 128:(c + 1) * 128, sl])

                    qps = proj_ps.tile([128, 512], F32, name="qps", tag="pps")
                    for c in range(KC):
                        nc.tensor.matmul(qps, wq_t[:, c * DH:(c + 1) * DH],
                                         xt[:, c],
                                         start=(c == 0), stop=(c == KC - 1))
                    nc.vector.tensor_scalar_add(qT[:, sl], qps, bqs)

                    kps = proj_ps.tile([128, 512], F32, name="kps", tag="pps")
                    for c in range(KC):
                        nc.tensor.matmul(kps, wk_t[:, c * DH:(c + 1) * DH],
                                         xt[:, c],
                                         start=(c == 0), stop=(c == KC - 1))
                    nc.vector.tensor_copy(kT[:, sl], kps)

                    # v like q/k (few big matmuls), then PE transposes to the
                    # [seq, hd] layout PV consumes
                    vps = proj_ps.tile([128, 512], F32, name="vps", tag="pps")
                    for c in range(KC):
                        nc.tensor.matmul(vps, wv_t[:, c * DH:(c + 1) * DH],
                                         xt[:, c],
                                         start=(c == 0), stop=(c == KC - 1))
                    vt = work.tile([128, 512], F32R, name="vt", tag="vt")
                    nc.vector.tensor_copy(vt, vps)
                    for blk in range(4):
                        g = t * 4 + blk
                        vtp = vt_ps.tile([128, 128], F32R, name="vtp",
                                         tag="vtp")
                        nc.tensor.transpose(
                            vtp, vt[:, blk * 128:(blk + 1) * 128], ident)
                        nc.vector.tensor_copy(
                            vv[:, 2 * g:2 * g + 2, 0:HD],
                            vtp.bitcast(F32).rearrange("p (a b) -> p a b",
                                                       b=HD))

            # ---- attention + output projection, software-pipelined ----
            with (
                tc.tile_pool(name="sc_ps", bufs=3, space="PSUM") as sc_ps,
                tc.tile_pool(name="oa_ps", bufs=3, space="PSUM") as oa_ps,
                tc.tile_pool(name="y_ps", bufs=2, space="PSUM") as y_ps,
            ):
                def kbs_of(qb):
                    return range(3 - min(qb + 1, 3), 3)

                def score_stage(qb):
                    """Transposed scores + exp + triangular masks -> expm."""
                    qsl = slice(qb * 128, (qb + 1) * 128)
                    expms = []
                    for h in range(2):
                        hs = slice(h * 64, (h + 1) * 64)
                        sc = sc_ps.tile([128, 384], F32, name="sc", tag="sc")
                        for kb in kbs_of(qb):
                            g = qb - 2 + kb
                            nc.tensor.matmul(
                                sc[:, COL[kb]:COL[kb] + 128],
                                kT[hs, g * 128:(g + 1) * 128],
                                qT[hs, qsl], start=True, stop=True)
                        expm = expp.tile([128, 384], F16, name="expm",
                                         tag=f"expm{h}")
                        # kb1 (cols 0:128) is fully visible; kb0/kb2 get the
                        # 0/1 triangular masks in place (exp never overflows:
                        # |score| < 3). For qb=0 only kb2 exists. NOTE: an
                        # all-gpsimd masking variant removed the DVE
                        # head-of-line block ahead of the finish chain and
                        # cut attention PE gaps 3x -- but the denser PE
                        # stream doubled power-throttle time and lost ~14us
                        # net. Keep the split.
                        eng = nc.vector if h == 0 else nc.gpsimd
                        if qb == 0:
                            nc.scalar.activation(
                                expm[:, 256:384], sc[:, 256:384],
                                mybir.ActivationFunctionType.Exp)
                            eng.tensor_mul(
                                expm[:, 256:384], expm[:, 256:384],
                                mask[:, 128:256])
                        else:
                            nc.scalar.activation(
                                expm, sc, mybir.ActivationFunctionType.Exp)
                            eng.tensor_mul(
                                expm[:, 128:384], expm[:, 128:384], mask)
                        expms.append(expm)
                    return expms

                def pv_stage(qb, expms):
                    """o_aug[q, h, 0:64] = P@V, col 64 = softmax row sum.

                    The oa tile is a full PSUM bank: cols 0:130 hold the two
                    o_aug blocks, cols 384:512 are scratch for the transpose.
                    """
                    oa = oa_ps.tile([128, 512], F32, name="oa", tag="oa")
                    oav = oa[:, 0:2 * VB].rearrange("p (a b) -> p a b", b=VB)
                    kbs = kbs_of(qb)
                    for h in range(2):
                        for kb in kbs:
                            g = qb - 2 + kb
                            nc.tensor.matmul(
                                oav[:, h],
                                expms[h][:, COL[kb]:COL[kb] + 128],
                                vv[:, 2 * g + h, :],
                                start=(kb == kbs[0]), stop=(kb == 2))
                    return oa

                def finish_norm(qb, oa):
                    """Normalize o and transpose back to [hd, q] (PE)."""
                    oav = oa[:, 0:2 * VB].rearrange("p (a b) -> p a b", b=VB)
                    invr = work.tile([128, 2], F32, name="invr", tag="invr")
                    nc.vector.reciprocal(invr, oav[:, :, 64:65])
                    onq = work.tile([128, 2, HD], F32R, name="onq", tag="onq")
                    for h in range(2):
                        nc.vector.tensor_scalar(
                            onq[:, h], oav[:, h, 0:HD], invr[:, h:h + 1], None,
                            mybir.AluOpType.mult)
                    nc.tensor.transpose(oa[:, 384:512].bitcast(F32R),
                                        onq.rearrange("p a b -> p (a b)"),
                                        ident)
                    onorm = work.tile([128, 128], F16, name="onorm",
                                      tag="onorm")
                    nc.vector.tensor_copy(onorm, oa[:, 384:512])
                    return onorm

                def finish_proj(qb, onorm, drain=False):
                    """Output projection of one query block + writeback."""
                    ysb = work.tile([128, 1024], F16, name="ysb", tag="ysb")
                    for half in range(2):
                        yp = y_ps.tile([128, 512], F32, name="yp", tag="yp")
                        nc.tensor.matmul(yp, onorm,
                                         wo_t[:, half * 512:(half + 1) * 512],
                                         start=True, stop=True)
                        ysl = slice(half * 512, (half + 1) * 512)
                        if half == 0:
                            nc.scalar.copy(ysb[:, ysl], yp)
                        else:
                            nc.vector.tensor_copy(ysb[:, ysl], yp)
                        if drain:
                            # end of kernel: other queues are idle, split the
                            # write 2 ways per half to shorten the last
                            # transfer's drain tail
                            for j in range(2):
                                ysl2 = slice(half * 512 + j * 256,
                                             half * 512 + (j + 1) * 256)
                                (nc.sync, nc.gpsimd, nc.scalar,
                                 nc.sync)[half * 2 + j].dma_start(
                                    y.ap()[qb * 128:(qb + 1) * 128, ysl2],
                                    ysb[:, ysl2])
                        else:
                            nc.sync.dma_start(
                                y.ap()[qb * 128:(qb + 1) * 128, ysl],
                                ysb[:, ysl])

                def finish(qb, oa, drain=False):
                    finish_proj(qb, finish_norm(qb, oa), drain)

                # 4-deep software pipeline: scores(qb) | PV(qb-1) | idle |
                # normalize+project(qb-3) keeps the PE fed while scalar and
                # vector engines chew the previous blocks. (A 5-deep variant
                # with near-zero PE gaps measured SLOWER: the denser PE
                # stream engages the 0.5-util power throttle.)
                hist = {}
                for qb in range(N_QB):
                    hist[qb] = [score_stage(qb), None]
                    if qb >= 1:
                        hist[qb - 1][1] = pv_stage(qb - 1, hist[qb - 1][0])
                    if qb >= 3:
                        finish(qb - 3, hist.pop(qb - 3)[1])
                hist[N_QB - 1][1] = pv_stage(N_QB - 1, hist[N_QB - 1][0])
                # drain: pipeline the last finishes stage-by-stage so their
                # engine chains overlap
                tail_qbs = tuple(range(N_QB - 3, N_QB))
                onorms = [finish_norm(qb, hist[qb][1]) for qb in tail_qbs]
                for qb, onorm in zip(tail_qbs, onorms):
                    finish_proj(qb, onorm, drain=True)

    if not nc.is_finalized():
        nc.finalize()
    return nc


def make_in_maps(x, Wq, bq, Wk, Wv, Wo):
    """Per-core input dict list; host does the fp16 casts and head sharding."""
    scale = 1.0 / float(np.sqrt(HD))
    xT = np.ascontiguousarray(np.asarray(x, np.float32)[0].T.astype(np.float16))
    in_maps = []
    for c in range(N_CORES):
        csl = slice(c * DH, (c + 1) * DH)
        in_maps.append({
            "xT": xT,
            "wq": np.ascontiguousarray(
                (np.asarray(Wq, np.float32)[:, csl] * scale).astype(np.float16)),
            "wk": np.ascontiguousarray(
                np.asarray(Wk, np.float32)[:, csl].astype(np.float16)),
            "wv": np.ascontiguousarray(
                np.asarray(Wv, np.float32)[:, csl].astype(np.float16)),
            "bq": np.ascontiguousarray(
                np.asarray(bq, np.float32)[csl] * scale),
            "wo": np.ascontiguousarray(
                np.asarray(Wo, np.float32)[csl, :].astype(np.float16)),
        })
    return in_maps


_NC_CACHE = None


def kernel(x, Wq, bq, Wk, bk, Wv, bv, Wo, bo, **_kw):
    global _NC_CACHE
    x = np.asarray(x, dtype=np.float32)
    B = x.shape[0]
    assert x.shape == (B, S, D) and B == 1

    in_maps = make_in_maps(x, Wq, bq, Wk, Wv, Wo)

    if _NC_CACHE is None:
        _NC_CACHE = build_kernel()
    res = run_bass_kernel_spmd(_NC_CACHE, in_maps, core_ids=list(range(N_CORES)))

    out = np.zeros((S, D), dtype=np.float32)
    for c in range(N_CORES):
        out += res.results[c]["y"].astype(np.float32)
    # host-side bias terms: bo plus the bv @ Wo constant row (see header)
    bv = np.asarray(bv, dtype=np.float32)
    bo = np.asarray(bo, dtype=np.float32)
    Wo = np.asarray(Wo, dtype=np.float32)
    out += (bv @ Wo + bo)[None, :]
    return out.reshape(1, S, D)
